# revision 17
# baseline (speedup 1.0000x reference)
"""Trainium2 Bass kernel for GQA attention (nn_Attention_40364102648437).

Problem: B=2, S=2048, HIDDEN=896, 14 q heads / 2 kv heads, head_dim 64,
RoPE (theta 1e6), causal softmax, o-projection.

Sharding (8 cores, SPMD): core = b*4 + kv*2 + half.
Each core owns one batch b, one kv head, and 4 q-head slots (7 q heads per
kv group are split 4+3; the last slot of the second half is a duplicate
whose wo rows are zeroed so its contribution vanishes). Every core computes
a full [S, HIDDEN] partial output (its heads' contribution through wo);
the host sums the 4 partials per batch.

Engine assignment (cost model: matmul time = output free size x cyc/row;
contraction and output partitions are free):
  - PE: projections + rope permutations (bf16, 1 cyc/row), scores in fp8
    DoubleRow (0.5 cyc/row: q/k tiles are [128, 2S] fp8 with a zeroed
    second half serving as the second k-subtile of the [64, 2, n] APs, so
    DoubleRow contracts real+0 with no partition-moving relayout), PV in
    bf16, o-projection in bf16, and the additive causal mask (two
    start=True matmuls seed the diagonal group's PSUM with {0, -30000}).
  - Activation: ONLY the softmax exp (~72us = the bottleneck; diagonal
    groups use a compact 768-column layout that skips the fully-masked
    quadrant) plus the per-chunk XBAR V-transpose DMAs on its queue.
  - DVE: rope sin/cos muls, PSUM evacuation (rope/o-proj copies, with the
    rope q copy folding the cos part in via scalar_tensor_tensor), the
    softmax tail (reciprocal + normalize muls).
  - Pool (gpsimd): SBUF-only work - 1/Z partition_broadcast, v re-stride
    copy, memsets. (GPSIMD cannot access PSUM on real HW.)
  - SP: every DMA (inputs up front in first-use order; HWDGE serializes
    at ~625ns per DMA and the DMA bus at ~22.5 B/ns).

RoPE runs as multiply-then-permute: qs = q*sin_signed on DVE, then a
permutation matmul on the PE does rotate-half (permKs/permKc also
duplicate k into both partition halves for the row-paired score matmuls).
No partition-moving DMAs in the rope path.

Softmax: V tiles carry a ones column so PV accumulates the denominator Z
in o_ab row 64; DVE reciprocal makes 1/Z, a small SBUF->SBUF DMA hops the
row to partition 0 (partition_broadcast reads physical partition 0), the
Pool broadcasts it, and DVE multiplies it in while copying the attention
output to SBUF (bf16) for the o-projection.

Emission schedule: one unified 8-bank PSUM pool (s_ps 4 + o_ab 2 + a
shared [128,512] tag for proj/rope/o-proj 2) avoids any phase barrier.
Chunks 0-1 (projections+rope per 512 q columns) are emitted eagerly;
chunks 2-3 drip between attention groups so every in-order engine queue
sees short-latency work. Attention superblock J consumes chunk J//2; the
diagonal k-group goes first in each pair so its exp pipelines like any
other group; o-projection of J is deferred to the end of J+1 so the PE
never waits on the slot-b restack DMA; the last J reads slot b straight
from the staging tile (no restack) and drains its copies via the
then-idle Act engine.

Hardware constraints found the hard way (sim passes, HW does not):
  - GPSIMD/Pool instructions cannot touch PSUM;
  - the XBAR transpose DMA needs a contiguous destination (a 65-pitch
    interleaved AP silently lands contiguously);
  - partition_broadcast replicates physical partition 0 at the AP's byte
    offset, not the AP's base partition;
  - TensorTensor SBUF operands must share the same start partition
    (walrus samePartitionsAll) and engines are lane-local;
  - matmul start=True clears has_written for the whole PSUM bank, so
    multi-region accumulation tiles get exactly one start per bank.
"""

import os

import numpy as np
import ml_dtypes

import concourse.bass as bass
import concourse.mybir as mybir
from concourse import bacc
from concourse.tile import TileContext
from concourse.masks import make_identity
from concourse.bass_utils import run_bass_kernel_spmd

F32 = mybir.dt.float32
BF16 = mybir.dt.bfloat16
FP8 = mybir.dt.float8e4
BF = ml_dtypes.bfloat16

HIDDEN = 896
HEAD_DIM = 64
B = 2
S = 2048
ROPE_THETA = 1000000.0
NH7 = HIDDEN // 128  # 7 hidden tiles
NKB = S // 128       # 16 key blocks
NJ = S // 256        # 8 query superblocks (256 q positions each)


def build_program():
    nc = bacc.Bacc("TRN2", target_bir_lowering=False, debug=False, num_devices=8)

    # host-pre-tiled: row ss*128+p holds [t, n] -> hs[b][ss*512+n, t*128+p]
    hsT = nc.dram_tensor("hsT", [4 * 128, NH7 * 512], BF16, kind="ExternalInput")
    wq4 = nc.dram_tensor("wq4", [HIDDEN, 256], BF16, kind="ExternalInput")
    bq4 = nc.dram_tensor("bq4", [2, 128], F32, kind="ExternalInput")
    wkv = nc.dram_tensor("wkv", [HIDDEN, 128], BF16, kind="ExternalInput")
    bkv = nc.dram_tensor("bkv", [1, 128], F32, kind="ExternalInput")
    wo4 = nc.dram_tensor("wo4", [256, HIDDEN], BF16, kind="ExternalInput")
    cosd = nc.dram_tensor("cosd", [64, S], BF16, kind="ExternalInput")
    sind = nc.dram_tensor("sind", [64, S], BF16, kind="ExternalInput")
    maskD = nc.dram_tensor("maskD", [128, 384], BF16, kind="ExternalInput")
    # rotate-half permutations as matmul weights: permQ does the half-swap
    # within each 64-row slot; permKs/permKc (cols 128:256 / 256:384, rows
    # 0:64) swap and duplicate k into both partition halves
    permD = nc.dram_tensor("permD", [128, 384], BF16, kind="ExternalInput")
    out_d = nc.dram_tensor("out", [S, HIDDEN], F32, kind="ExternalOutput")

    EXP = mybir.ActivationFunctionType.Exp

    with TileContext(nc) as tc:
        with (
            tc.tile_pool(name="const", bufs=1) as cpool,
            tc.tile_pool(name="big", bufs=1) as bigpool,
            tc.tile_pool(name="psu", bufs=2, space="PSUM") as pspool,
            tc.tile_pool(name="hst", bufs=4) as hpool,
            tc.tile_pool(name="swp", bufs=4) as swpool,
            tc.tile_pool(name="esb", bufs=6) as epool,
            tc.tile_pool(name="rzs", bufs=4) as rzpool,
            tc.tile_pool(name="osb", bufs=4) as obpool,
        ):
            # ---- constants, issued in first-use order (DMAs serialize on
            # the single HWDGE device and the DMA bus)
            wkv_sb = cpool.tile([128, NH7 * 128], BF16)
            nc.sync.dma_start(
                out=wkv_sb[:].rearrange("p (t f) -> p t f", t=NH7),
                in_=wkv.rearrange("(t p) f -> p t f", p=128),
            )
            bkv_sb = cpool.tile([128, 1], F32)
            nc.sync.dma_start(out=bkv_sb[:], in_=bkv.rearrange("a p -> p a"))
            perm_sb = cpool.tile([128, 384], BF16)
            nc.sync.dma_start(out=perm_sb[:], in_=permD[:])
            wq_sb = cpool.tile([128, NH7 * 256], BF16)
            bq_sb = cpool.tile([128, 2], F32)
            cos_sb = cpool.tile([128, S], BF16)
            sin_sb = cpool.tile([128, S], BF16)
            wo_sb = cpool.tile([128, 2 * HIDDEN], BF16)
            mask_sb = cpool.tile([128, 384], BF16)
            identb = cpool.tile([128, 128], BF16)
            make_identity(nc, identb[:])
            # force the Exp activation table load off the critical path
            warm = cpool.tile([1, 8], F32)
            nc.vector.memset(warm[:], 0.0)
            nc.scalar.activation(warm[:], warm[:], EXP, bias=0.0, scale=1.0)

            def load_consts_pre():
                nc.sync.dma_start(
                    out=wq_sb[:].rearrange("p (t f) -> p t f", t=NH7),
                    in_=wq4.rearrange("(t p) f -> p t f", p=128),
                )
                nc.sync.dma_start(out=bq_sb[:], in_=bq4.rearrange("a p -> p a"))
                # full-height tables loaded as two copies of the 64-row
                # DRAM table (same bytes as one 128-row load, and DRAM
                # sources avoid an SBUF->SBUF dup dependency bubble)
                nc.sync.dma_start(out=cos_sb[0:64, :], in_=cosd[:])
                nc.sync.dma_start(out=cos_sb[64:128, :], in_=cosd[:])
                nc.sync.dma_start(out=sin_sb[0:64, :], in_=sind[:])
                nc.sync.dma_start(out=sin_sb[64:128, :], in_=sind[:])
                nc.sync.dma_start(out=mask_sb[:], in_=maskD[:])

            def load_consts_mid():
                nc.sync.dma_start(
                    out=wo_sb[:].rearrange("p (t f) -> p t f", t=2),
                    in_=wo4.rearrange("(t p) f -> p t f", p=128),
                )

            # ---- persistent activations (all bf16)
            kvT = bigpool.tile([128, S], BF16)   # rows 0-63 k, 64-127 vT
            qA = bigpool.tile([128, S], BF16)
            qB = bigpool.tile([128, S], BF16)
            # rope'd q/k in fp8 for DoubleRow score matmuls (0.5 cyc/row):
            # cols S..2S stay zero and act as the second k-subtile, so the
            # [64, 2, n] DoubleRow APs contract (real + 0) with no
            # partition-moving relayout
            kdr = bigpool.tile([128, 2 * S], FP8)
            qAr = bigpool.tile([128, 2 * S], FP8)
            qBr = bigpool.tile([128, 2 * S], FP8)
            nc.gpsimd.memset(kdr[:, S:], 0.0)
            nc.gpsimd.memset(qAr[:, S:], 0.0)
            nc.gpsimd.memset(qBr[:, S:], 0.0)
            v_sb = bigpool.tile([128, NKB * 65], BF16)
            vD = bigpool.tile([128, NKB * 64], BF16)  # contiguous XBAR dst
            aoT0 = bigpool.tile([128, S], BF16)
            aoT1 = bigpool.tile([128, S], BF16)

            # ---- attention helpers ------------------------------------
            def emit_oproj(J, use_act=False):
                for qb in (2 * J, 2 * J + 1):
                    ob = obpool.tile([128, HIDDEN], F32)
                    for half in range(2):
                        hsl = slice(half * 448, (half + 1) * 448)
                        f_ps = pspool.tile([128, 512], F32, tag="b512")
                        for ft in range(2):
                            aoTt = (aoT0, aoT1)[ft]
                            wsl = slice(ft * HIDDEN + half * 448,
                                        ft * HIDDEN + (half + 1) * 448)
                            nc.tensor.matmul(
                                f_ps[:, 0:448],
                                aoTt[:, qb * 128 : (qb + 1) * 128],
                                wo_sb[:, wsl],
                                start=(ft == 0), stop=(ft == 1),
                            )
                        if use_act:
                            # tail: Act is idle after the last exp
                            nc.scalar.copy(ob[:, hsl], f_ps[:, 0:448])
                        else:
                            nc.vector.tensor_copy(ob[:, hsl], f_ps[:, 0:448])
                        nc.sync.dma_start(
                            out=out_d[qb * 128 : (qb + 1) * 128, hsl],
                            in_=ob[:, hsl],
                        )

            def emit_J(J):
                # the two pairs' group loops are interleaved (zig-zag) so
                # the exp stream never drains at a pair boundary; each pair
                # keeps its own flipped-PV accumulator [128 q, 4*65] (head x
                # q-half x {64 v dims + Z col}) and PV software pipeline
                qsl = slice(J * 256, (J + 1) * 256)
                o_ab0 = pspool.tile([128, 260], F32, tag="oq")
                o_ab1 = pspool.tile([128, 260], F32, tag="oq")
                o_abs = [o_ab0, o_ab1]
                pend = [None, None]
                gorder = [J] + list(range(J))
                for gi, g in enumerate(gorder):
                    diag = g == J
                    for pair in range(2):
                        qt = (qAr, qBr)[pair]
                        o_ab = o_abs[pair]
                        s_ps = pspool.tile([128, 1024], F32, tag="sps")
                        if diag:
                            # compact bank-aligned diagonal layout: per
                            # half, cols h*512+0:256 = kb 2J vs all 256 q,
                            # cols h*512+256:384 = kb 2J+1 vs the upper
                            # 128 q; the fully-masked quadrant is never
                            # computed. Additive mask seeded on the PE.
                            for half in range(2):
                                base = half * 512
                                hrows = slice(half * 64, (half + 1) * 64)
                                kdr3 = kdr[hrows, :].rearrange(
                                    "p (j s) -> p j s", j=2
                                )
                                qt3 = qt[hrows, :].rearrange(
                                    "p (j s) -> p j s", j=2
                                )
                                nc.tensor.matmul(
                                    s_ps[:, base : base + 384],
                                    identb[:],
                                    mask_sb[:],
                                    start=True,
                                    stop=False,
                                    skip_group_check=True,
                                )
                                nc.tensor.matmul(
                                    s_ps[:, base : base + 256],
                                    kdr3[:, :, 2 * J * 128 : (2 * J + 1) * 128],
                                    qt3[:, :, qsl],
                                    start=False,
                                    stop=True,
                                    perf_mode=mybir.MatmulPerfMode.DoubleRow,
                                    skip_group_check=True,
                                )
                                nc.tensor.matmul(
                                    s_ps[:, base + 256 : base + 384],
                                    kdr3[:, :, (2 * J + 1) * 128 : (2 * J + 2) * 128],
                                    qt3[:, :, J * 256 + 128 : (J + 1) * 256],
                                    start=False,
                                    stop=True,
                                    perf_mode=mybir.MatmulPerfMode.DoubleRow,
                                    skip_group_check=True,
                                )
                            e_sb = epool.tile([128, 1024], BF16)
                            nc.scalar.activation(
                                e_sb[:].rearrange("p (b f) -> p b f", b=2)[
                                    :, :, 0:384
                                ],
                                s_ps[:].rearrange("p (b f) -> p b f", b=2)[
                                    :, :, 0:384
                                ],
                                EXP,
                                bias=0.0,
                                scale=0.125,
                            )
                        else:
                            for i, kb in enumerate((2 * g, 2 * g + 1)):
                                for half in range(2):
                                    # concurrent row-group pair must write
                                    # different PSUM banks
                                    seg = half * 512 + i * 256
                                    hrows = slice(half * 64, (half + 1) * 64)
                                    lhs3 = kdr[hrows, :].rearrange(
                                        "p (j s) -> p j s", j=2
                                    )[:, :, kb * 128 : (kb + 1) * 128]
                                    rhs3 = qt[hrows, :].rearrange(
                                        "p (j s) -> p j s", j=2
                                    )[:, :, qsl]
                                    nc.tensor.matmul(
                                        s_ps[:, seg : seg + 256],
                                        lhs3,
                                        rhs3,
                                        start=True,
                                        stop=True,
                                        perf_mode=mybir.MatmulPerfMode.DoubleRow,
                                        skip_group_check=True,
                                    )
                            e_sb = epool.tile([128, 1024], BF16)
                            nc.scalar.activation(
                                e_sb[:], s_ps[:], EXP, bias=0.0, scale=0.125
                            )
                        if pend[pair] is not None:
                            _emit_pv(nc, o_ab, v_sb, *pend[pair])
                        pend[pair] = (e_sb, g, gi == 0, gi == J, diag)
                    # chunk production is dripped only into the late,
                    # Act-bound superblocks where the PE has slack
                    drip(1 if J >= 3 else 0)
                for pair in range(2):
                    _emit_pv(nc, o_abs[pair], v_sb, *pend[pair])
                # o-projection of J-1 next: its inputs are long ready, so
                # it fills the PE while the pair tails' DVE chains run
                # (and leaves only oproj(J) in the program drain)
                if J > 0:
                    emit_oproj(J - 1)
                for pair in range(2):
                    aoT = (aoT0, aoT1)[pair]
                    o_ab = o_abs[pair]

                    # per (head, q-half): Z sits in col 64 of the 65-col
                    # group; normalize with a per-partition scalar mult while
                    # casting to bf16, then a PE transpose restores the
                    # [head-dim, q] layout o-proj wants (both head slots
                    # stack into one PSUM tile, one DVE evac per q-half)
                    for qh in range(2):
                        # transpose writes bf16 (must match lhsT dtype), so
                        # view the fp32 PSUM tile's first 64 cols as bf16
                        t_ps = pspool.tile([128, 512], F32, tag="b512")
                        for head in range(2):
                            base = (head * 2 + qh) * 65
                            rz = rzpool.tile([128, 1], F32)
                            nc.vector.reciprocal(
                                rz[:], o_ab[:, base + 64 : base + 65]
                            )
                            nrm = swpool.tile([128, 64], BF16, tag="nrm")
                            nc.vector.tensor_scalar_mul(
                                nrm[:], o_ab[:, base : base + 64], rz[:, 0:1]
                            )
                            nc.tensor.matmul(
                                t_ps[
                                    head * 64 : (head + 1) * 64, 0:64
                                ].bitcast(BF16),
                                nrm[:],
                                identb[:],
                                is_transpose=True,
                                skip_group_check=True,
                            )
                        nc.vector.tensor_copy(
                            aoT[:, J * 256 + qh * 128 : J * 256 + (qh + 1) * 128],
                            t_ps[:, 0:64].bitcast(BF16),
                        )
                    drip(2 if J >= 3 else 0)


            # ---- staged emission via a production work-queue ---------
            # All input DMAs are issued up front in first-use order (the SP
            # queue and HWDGE serialize; attention-time restack/out DMAs
            # must queue behind none of them). Chunk production (proj, rope,
            # V transpose) is split into micro-ops that are dripped between
            # attention groups, so each engine's in-order queue always sees
            # short-latency work and cross-engine chains never block.
            hs_tiles = []
            for ss in range(4):
                hs_t = hpool.tile([128, NH7 * 512], BF16)
                hs_tiles.append(hs_t)
                if ss == 0:
                    # split so the first kv matmuls start sooner
                    nc.sync.dma_start(
                        out=hs_t[:, 0 : 3 * 512], in_=hsT[0:128, 0 : 3 * 512]
                    )
                    nc.sync.dma_start(
                        out=hs_t[:, 3 * 512 :], in_=hsT[0:128, 3 * 512 :]
                    )
                    load_consts_pre()
                else:
                    if ss == 2:
                        load_consts_mid()
                    nc.sync.dma_start(
                        out=hs_t[:], in_=hsT[ss * 128 : (ss + 1) * 128, :]
                    )

            def mk_proj(ss, w_sb, wofs, wpitch, dst, bias, bcol):
                def fn():
                    ssl = slice(ss * 512, (ss + 1) * 512)
                    ps = pspool.tile([128, 512], F32, tag="b512")
                    for h in range(NH7):
                        nc.tensor.matmul(
                            ps[:],
                            w_sb[:, h * wpitch + wofs : h * wpitch + wofs + 128],
                            hs_tiles[ss][:, h * 512 : (h + 1) * 512],
                            start=(h == 0),
                            stop=(h == NH7 - 1),
                        )
                    if ss < 2:
                        # warmup chunks: bias-add on the idle Act engine so
                        # the DVE in-order queue reaches the rope ops sooner
                        nc.scalar.activation(
                            dst[:, ssl], ps[:],
                            mybir.ActivationFunctionType.Identity,
                            bias=bias[:, bcol : bcol + 1], scale=1.0,
                        )
                    else:
                        nc.vector.tensor_scalar_add(
                            dst[:, ssl], ps[:], bias[:, bcol : bcol + 1]
                        )
                return fn

            def mk_krope(ss):
                def fn():
                    ssl = slice(ss * 512, (ss + 1) * 512)
                    ks = swpool.tile([64, 512], BF16, tag="ks")
                    nc.vector.tensor_mul(ks[:], kvT[0:64, ssl], sin_sb[0:64, ssl])
                    nc.vector.tensor_mul(
                        kvT[0:64, ssl], kvT[0:64, ssl], cos_sb[0:64, ssl]
                    )
                    r_ps = pspool.tile([128, 512], F32, tag="b512")
                    nc.tensor.matmul(
                        r_ps[:], perm_sb[0:64, 128:256], ks[:],
                        start=True, stop=False,
                    )
                    nc.tensor.matmul(
                        r_ps[:], perm_sb[0:64, 256:384], kvT[0:64, ssl],
                        start=False, stop=True,
                    )
                    if ss < 2:
                        nc.scalar.copy(kdr[:, ssl], r_ps[:])  # fp8 out
                    else:
                        nc.vector.tensor_copy(kdr[:, ssl], r_ps[:])  # fp8 out
                return fn

            def mk_qrope(ss, t, tr, on_pool):
                def fn():
                    ssl = slice(ss * 512, (ss + 1) * 512)
                    qs = swpool.tile([128, 512], BF16, tag="qs")
                    nc.vector.tensor_mul(qs[:], t[:, ssl], sin_sb[:, ssl])
                    nc.vector.tensor_mul(t[:, ssl], t[:, ssl], cos_sb[:, ssl])
                    r_ps = pspool.tile([128, 512], F32, tag="b512")
                    nc.tensor.matmul(
                        r_ps[:], perm_sb[:, 0:128], qs[:],
                        start=True, stop=True,
                    )
                    # tr = swap(q*sin) + q*cos, cast to fp8, in one op
                    nc.vector.scalar_tensor_tensor(
                        tr[:, ssl], r_ps[:], 1.0, t[:, ssl],
                        op0=mybir.AluOpType.mult,
                        op1=mybir.AluOpType.add,
                    )
                return fn

            def mk_vt(ss):
                def fn():
                    # one XBAR transpose DMA [64, 512] -> [128, 4, 64] on
                    # the Activation HWDGE queue. The XBAR needs a
                    # CONTIGUOUS destination (a 65-pitch interleaved AP
                    # silently lands contiguously on HW), so transpose into
                    # vD then re-stride into v_sb on the Pool engine.
                    vdst = vD[
                        :, ss * 4 * 64 : (ss + 1) * 4 * 64
                    ].rearrange("p (t f) -> p t f", f=64)
                    nc.scalar.dma_start(
                        out=vdst,
                        in_=kvT[64:128, ss * 512 : (ss + 1) * 512],
                        transpose=True,
                    )
                    nc.gpsimd.tensor_copy(
                        v_sb[
                            :, ss * 4 * 65 : (ss + 1) * 4 * 65
                        ].rearrange("p (t f) -> p t f", f=65)[:, :, 0:64],
                        vdst,
                    )
                    nc.gpsimd.memset(
                        v_sb[:, ss * 4 * 65 + 64 : (ss + 1) * 4 * 65 : 65], 1.0
                    )
                return fn

            def chunk_ops(ss):
                return [
                    mk_proj(ss, wkv_sb, 0, 128, kvT, bkv_sb, 0),
                    mk_proj(ss, wq_sb, 0, 256, qA, bq_sb, 0),
                    mk_proj(ss, wq_sb, 128, 256, qB, bq_sb, 1),
                    mk_krope(ss),
                    mk_vt(ss),
                    mk_qrope(ss, qA, qAr, True),
                    mk_qrope(ss, qB, qBr, False),
                ]

            # drip queue: (chunk, fn) in order; need(c) force-drains all
            # micro-ops of chunks <= c before the attention that reads them
            pending = []

            def drip(n):
                for _ in range(min(n, len(pending))):
                    pending.pop(0)[1]()

            def need(c):
                while pending and pending[0][0] <= c:
                    pending.pop(0)[1]()

            for fn in chunk_ops(0):
                fn()
            for fn in chunk_ops(1):
                fn()
            for c in (2, 3):
                pending.extend((c, fn) for fn in chunk_ops(c))
            for J in range(NJ):
                # superblock J reads q/k chunks up to J // 2
                need(min(3, J // 2))
                emit_J(J)
            emit_oproj(NJ - 1, use_act=True)

            if os.environ.get("K_DEBUG"):
                # overwrite out rows with intermediates for HW bisection
                nc.sync.dma_start(
                    out=out_d[0:128, :], in_=kdr[:, 0 : 896 * 4].bitcast(F32)
                )
                nc.sync.dma_start(
                    out=out_d[128:256, :], in_=qAr[:, 0 : 896 * 4].bitcast(F32)
                )
                nc.sync.dma_start(
                    out=out_d[256:384, 0:520],
                    in_=v_sb[:, 0:1040].bitcast(F32),
                )
                nc.sync.dma_start(
                    out=out_d[384:512, 0:448],
                    in_=aoT0[:, 0:896].bitcast(F32),
                )

    nc.compile()
    return nc


def _emit_pv(nc, o_ab, v_sb, e_sb, g, first, last, diag):
    """Flipped PV accumulation for one exp'd group (k-blocks 2g, 2g+1):
    lhsT = e [128 k, 128 q], rhs = v [128 k, 65] -> o[128 q, 65] so the
    matmul cost is 65 output columns instead of 256 (stationary operand
    and contraction are free). Z accumulates in col 64 via the v ones
    column. Diagonal groups: kb 2g covers both q-halves, kb 2g+1 only the
    upper one (compact e layout cols 256:384 per head)."""
    for i, kb in enumerate((2 * g, 2 * g + 1)):
        for head in range(2):
            for qh in range(2):
                if diag and i == 1 and qh == 0:
                    continue  # fully-masked quadrant, never computed
                if diag and i == 1:
                    src = head * 512 + 256
                else:
                    src = head * 512 + i * 256 + qh * 128
                dst = (head * 2 + qh) * 65
                # one accumulation group for the whole o_ab tile: start=True
                # clears has_written for the entire PSUM bank, so only the
                # very first matmul may set it
                nc.tensor.matmul(
                    o_ab[:, dst : dst + 65],
                    e_sb[:, src : src + 128],
                    v_sb[:, kb * 65 : (kb + 1) * 65],
                    start=(first and i == 0 and head == 0 and qh == 0),
                    stop=(last and i == 1 and head == 1 and qh == 1),
                    skip_group_check=True,
                )


def _rope_tables():
    """[64, S] cos and pre-swap sign-folded sin: rope(x) = x*cos +
    perm(x*sind) with perm the half-swap, so sind rows 32:64 carry the
    minus sign (they land in rows 0:32 after the swap)."""
    inv_freq = 1.0 / (
        ROPE_THETA ** (np.arange(0, HEAD_DIM, 2, dtype=np.float32) / HEAD_DIM)
    )
    t = np.arange(S, dtype=np.float32)
    freqs = np.outer(t, inv_freq)  # [S, 32]
    emb = np.concatenate([freqs, freqs], axis=-1)  # [S, 64]
    cosd = np.cos(emb).T.astype(np.float32)  # [64, S]
    sind = np.sin(emb).T.astype(np.float32)
    sind[32:64] = -sind[32:64]
    return np.ascontiguousarray(cosd.astype(BF)), np.ascontiguousarray(
        sind.astype(BF)
    )


def _perms():
    """[128, 384] bf16: permQ | permKs | permKc (matmul lhsT layout:
    lhsT[p, i] = 1 selects input partition p for output partition i)."""
    p = np.zeros((128, 384), np.float32)
    for i in range(128):
        blk, d = (i // 64) * 64, i % 64
        p[blk + (d + 32) % 64, i] = 1.0          # permQ: half-swap per slot
        p[(d + 32) % 64, 128 + i] = 1.0          # permKs: swap + duplicate
        p[d, 256 + i] = 1.0                      # permKc: duplicate
    return np.ascontiguousarray(p.astype(BF))


def _masks():
    """Additive causal mask [128, 384] bf16 for the compact diagonal
    layout: cols 0:256 mask kb 2J against all 256 q, cols 256:384 mask
    kb 2J+1 against the upper 128 q (0 where k <= q, -30000 elsewhere;
    exp(0.125 * -30000) underflows to exactly 0)."""
    kp = np.arange(128)[:, None]
    m1 = np.where(kp <= np.arange(256)[None, :], 0.0, -30000.0)
    m2 = np.where(kp <= np.arange(128)[None, :], 0.0, -30000.0)
    return np.ascontiguousarray(
        np.concatenate([m1, m2], axis=1).astype(BF)
    )  # [128, 384]


def _tile_hsT(hsT):
    """[896, 2048] -> [512, 3584]: row ss*128+p = concat over t of
    hsT[t*128+p, ss*512:(ss+1)*512], matching the SBUF projection layout."""
    out = np.empty((4 * 128, NH7 * 512), BF)
    for ss in range(4):
        blk = hsT[:, ss * 512 : (ss + 1) * 512].reshape(NH7, 128, 512)
        out[ss * 128 : (ss + 1) * 128, :] = (
            blk.transpose(1, 0, 2).reshape(128, NH7 * 512).astype(BF)
        )
    return np.ascontiguousarray(out)


_CONST_CACHE = None


def make_in_maps(hidden_states, wq, bq, wk, bk, wv, bv, wo):
    global _CONST_CACHE
    if _CONST_CACHE is None:
        cosd, sind = _rope_tables()
        _CONST_CACHE = (cosd, sind, _masks(), _perms())
    cosd, sind, maskD, permD = _CONST_CACHE
    # the tiled hidden states are shared by the 4 cores of a batch
    hs_tiled = [_tile_hsT(hidden_states[b].T) for b in range(B)]
    in_maps = []
    for core in range(8):
        b, kv, half = core // 4, (core % 4) // 2, core % 2
        if half == 0:
            slots = [kv * 7 + 0, kv * 7 + 1, kv * 7 + 2, kv * 7 + 3]
            dup = []
        else:
            slots = [kv * 7 + 4, kv * 7 + 5, kv * 7 + 6, kv * 7 + 3]
            dup = [3]
        cols = np.concatenate([np.arange(h * 64, (h + 1) * 64) for h in slots])
        wq4 = np.ascontiguousarray(wq[:, cols].astype(BF))
        bq4 = np.ascontiguousarray(bq[cols].reshape(2, 128))
        wkv = np.ascontiguousarray(
            np.concatenate(
                [wk[:, kv * 64 : (kv + 1) * 64], wv[:, kv * 64 : (kv + 1) * 64]],
                axis=1,
            ).astype(BF)
        )
        bkv = np.ascontiguousarray(
            np.concatenate(
                [bk[kv * 64 : (kv + 1) * 64], bv[kv * 64 : (kv + 1) * 64]]
            ).reshape(1, 128)
        )
        wo4 = wo[cols, :].copy()
        for d in dup:
            wo4[d * 64 : (d + 1) * 64, :] = 0.0
        in_maps.append(
            {
                "hsT": hs_tiled[b],
                "wq4": wq4,
                "bq4": bq4,
                "wkv": wkv,
                "bkv": bkv,
                "wo4": np.ascontiguousarray(wo4.astype(BF)),
                "cosd": cosd,
                "sind": sind,
                "maskD": maskD,
                "permD": permD,
            }
        )
    return in_maps


_NC_CACHE = None


def _get_program():
    global _NC_CACHE
    if _NC_CACHE is None:
        _NC_CACHE = build_program()
    return _NC_CACHE


def kernel(hidden_states, wq, bq, wk, bk, wv, bv, wo):
    hidden_states = np.asarray(hidden_states, np.float32)
    wq = np.asarray(wq, np.float32)
    bq = np.asarray(bq, np.float32)
    wk = np.asarray(wk, np.float32)
    bk = np.asarray(bk, np.float32)
    wv = np.asarray(wv, np.float32)
    bv = np.asarray(bv, np.float32)
    wo = np.asarray(wo, np.float32)

    nc = _get_program()
    in_maps = make_in_maps(hidden_states, wq, bq, wk, bk, wv, bv, wo)
    res = run_bass_kernel_spmd(nc, in_maps, list(range(8)))
    out = np.zeros((B, S, HIDDEN), np.float32)
    for core in range(8):
        out[core // 4] += res.results[core]["out"]
    return out

